# revision 1
# baseline (speedup 1.0000x reference)
"""Beran survival estimator (nn_Beran) — Trainium2 Bass kernel.

kernel(**inputs) takes the FULL inputs (c_p [16,256,8] f32, c_in [8192,16] int,
delta_in [8192] f32, bandwidth [1] f32) and returns (surv_func, surv_steps),
both [256, 8192] f32, matching reference.reference().

Strategy (8 NeuronCores, data-parallel over batch B=256 -> 32 rows/core):
  - per-core layout: partitions p = s*32 + b (s = one of 4 N-segments, b =
    local batch row), free axis f in [0,2048)
  - the softmax/one-hot gather term G[b,n] = sum_c p[c,b,c_in[n,c]] is a
    single 128-deep matmul against a one-hot matrix built on-device
  - cumsums via the DVE tensor_tensor_scan, cross-segment offsets via tiny
    constant-mask matmuls on the PE
"""
import os
import sys

import numpy as np

for _p in ("/opt/trn_rl_repo", os.path.expanduser("~/.axon_site/_ro/trn_rl_repo")):
    if os.path.isdir(_p) and _p not in sys.path:
        sys.path.insert(0, _p)

import ml_dtypes

import concourse.bacc as bacc
import concourse.bass as bass
import concourse.mybir as mybir
import concourse.tile as tile
from concourse.bass_utils import run_bass_kernel_spmd

f32 = mybir.dt.float32
bf16 = mybir.dt.bfloat16
fp16 = mybir.dt.float16

C, B, K = 16, 256, 8
N = 8192
NCORES = 8
Bc = B // NCORES          # 32
S = 4
F = N // S                # 2048
CH = 512                  # chunk width
NCH = F // CH             # 4
CK = C * K                # 128

TOLF = np.float32(1e-8 + 1e-5)
C1MTOL = np.float32(1.0 - float(TOLF))
EPS = np.float32(1e-13)
Alu = mybir.AluOpType
Act = mybir.ActivationFunctionType


def _consts():
    si = np.arange(128) // 32
    bi = np.arange(128) % 32
    same_b = bi[:, None] == bi[None, :]
    M1 = (same_b & (si[:, None] < si[None, :])).astype(np.float32)
    M2 = same_b.astype(np.float32)
    M3 = (same_b & (si[:, None] == si[None, :] - 1)).astype(np.float32)
    M4 = (same_b & (si[:, None] == 3)).astype(np.float32)
    ck = np.arange(128)
    Mk = ((ck % 16)[:, None] == (ck % 16)[None, :]).astype(np.float32)
    kvec = (ck // 16).astype(np.float32).reshape(128, 1)
    e0col = (si == 0).astype(np.float32).reshape(128, 1)
    return np.concatenate([M1, M2, M3, M4, Mk, kvec, e0col], axis=1)  # [128, 642]


def build_nc():
    from contextlib import ExitStack

    nc = bacc.Bacc()

    cpT_d = nc.dram_tensor("cpT", [CK, Bc], f32, kind="ExternalInput")
    cpB_d = nc.dram_tensor("cpB", [Bc, CK], f32, kind="ExternalInput")
    cinT_d = nc.dram_tensor("cinT", [C, N], bf16, kind="ExternalInput")
    delta_d = nc.dram_tensor("delta", [N], bf16, kind="ExternalInput")
    band_d = nc.dram_tensor("band", [1, 1], f32, kind="ExternalInput")
    surv_d = nc.dram_tensor("surv", [Bc, N], f32, kind="ExternalOutput")
    steps_d = nc.dram_tensor("steps", [Bc, N], f32, kind="ExternalOutput")

    call_d = nc.inline_tensor(_consts(), "constall")

    with tile.TileContext(nc) as tc, ExitStack() as ctx:
        cons = ctx.enter_context(tc.tile_pool(name="cons", bufs=1))
        bigp = ctx.enter_context(tc.tile_pool(name="bigp", bufs=1))
        smal = ctx.enter_context(tc.tile_pool(name="smal", bufs=1))
        gps = ctx.enter_context(tc.tile_pool(name="gps", bufs=1, space="PSUM"))
        sps = ctx.enter_context(tc.tile_pool(name="sps", bufs=3, space="PSUM"))

        dma = nc.sync.dma_start

        # consts first (tiny, everything needs them), then the cin replicate
        call_t = cons.tile([128, 642], f32, tag="call")
        dma(out=call_t, in_=call_d[:, :])

        cin_rep = bigp.tile([128, N], bf16, tag="cinrep")
        for s in range(S):
            eng = dma if s < 2 else nc.scalar.dma_start
            eng(out=cin_rep[:, s * F:(s + 1) * F],
                in_=bass.AP(tensor=cinT_d, offset=s * F,
                            ap=[[0, 8], [N, C], [1, F]]))
        M1 = call_t[:, 0:128]
        M2 = call_t[:, 128:256]
        M3 = call_t[:, 256:384]
        M4 = call_t[:, 384:512]
        Mk = call_t[:, 512:640]
        kvec = call_t[:, 640:641]
        e0col = call_t[:, 641:642]

        # ---- softmax P in (ck, b) layout, full fp32 ----
        cpT = smal.tile([CK, Bc], f32, tag="cpT")
        dma(out=cpT, in_=cpT_d[:, :])
        E = smal.tile([CK, Bc], f32, tag="E")
        nc.scalar.activation(out=E, in_=cpT, func=Act.Exp)
        # dummy matmul so the PE observes the const-DMA semaphore before the
        # real denominator matmul (PE LDW carries only one wait slot)
        dum_ps = sps.tile([1, 1], f32, tag="sp")
        nc.tensor.matmul(dum_ps, Mk[:, 0:1], Mk[:, 0:1], start=True, stop=True)
        den_ps = sps.tile([128, Bc], f32, tag="sp")
        nc.tensor.matmul(den_ps, Mk, E, start=True, stop=True)
        rden = smal.tile([128, Bc], f32, tag="rden")
        nc.vector.reciprocal(out=rden, in_=den_ps)
        P = smal.tile([CK, Bc], f32, tag="P")
        nc.vector.tensor_tensor(out=P, in0=E, in1=rden, op=Alu.mult)
        Phi = smal.tile([CK, Bc], fp16, tag="Phi")
        nc.vector.tensor_copy(out=Phi, in_=P)
        Phi32 = smal.tile([CK, Bc], f32, tag="Phi32")
        nc.vector.tensor_copy(out=Phi32, in_=Phi)
        Plo = smal.tile([CK, Bc], fp16, tag="Plo")
        nc.vector.tensor_tensor(out=Plo, in0=P, in1=Phi32, op=Alu.subtract)

        # DVE observer of the const DMA so H waits on one semaphore only
        obs1 = smal.tile([128, 1], f32, tag="obs1")
        nc.vector.tensor_copy(out=obs1, in_=call_t[:, 641:642])

        # ---- H one-hot [128, N] fp16 (DVE 4x) ----
        H = bigp.tile([128, N], fp16, tag="H")
        for s in range(S):
            nc.vector.tensor_scalar(out=H[:, s * F:(s + 1) * F],
                                    in0=cin_rep[:, s * F:(s + 1) * F],
                                    scalar1=kvec, scalar2=None, op0=Alu.is_equal)

        # ---- A[b] = sum_ck p^2, replicated over (s,b) partitions ----
        cpR = smal.tile([128, C, K], f32, tag="cpR")
        nc.scalar.dma_start(
            out=cpR, in_=bass.AP(tensor=cpB_d, offset=0,
                                 ap=[[0, S], [CK, Bc], [K, C], [1, K]]))
        t2 = smal.tile([128, C, K], f32, tag="t2")
        nc.scalar.activation(out=t2, in_=cpR, func=Act.Exp)
        v = smal.tile([128, C], f32, tag="v")
        nc.vector.reduce_sum(out=v, in_=t2, axis=mybir.AxisListType.X)
        t2sq = smal.tile([128, C, K], f32, tag="t2sq")
        nc.vector.tensor_tensor(out=t2sq, in0=t2, in1=t2, op=Alu.mult)
        u = smal.tile([128, C], f32, tag="u")
        nc.vector.reduce_sum(out=u, in_=t2sq, axis=mybir.AxisListType.X)
        rv = smal.tile([128, C], f32, tag="rv")
        nc.vector.reciprocal(out=rv, in_=v)
        w1 = smal.tile([128, C], f32, tag="w1")
        nc.vector.tensor_tensor(out=w1, in0=u, in1=rv, op=Alu.mult)
        w2 = smal.tile([128, C], f32, tag="w2")
        nc.vector.tensor_tensor(out=w2, in0=w1, in1=rv, op=Alu.mult)
        A = smal.tile([128, 1], f32, tag="A")
        nc.vector.reduce_sum(out=A, in_=w2, axis=mybir.AxisListType.X)

        # ---- bandwidth -> escale / ebias (per-partition) ----
        band_r = smal.tile([128, 1], f32, tag="band_r")
        nc.gpsimd.dma_start(out=band_r,
                            in_=bass.AP(tensor=band_d, offset=0,
                                        ap=[[0, 128], [1, 1]]))
        bwc = smal.tile([128, 1], f32, tag="bwc")
        nc.vector.tensor_scalar(out=bwc, in0=band_r, scalar1=0.1, scalar2=10.0,
                                op0=Alu.max, op1=Alu.min)
        rbw = smal.tile([128, 1], f32, tag="rbw")
        nc.vector.reciprocal(out=rbw, in_=bwc)
        nrbw_b = smal.tile([128, 1], f32, tag="nrbwb")
        nc.scalar.mul(out=nrbw_b, in_=rbw, mul=-1.0)
        escale = smal.tile([128, 1], f32, tag="escale")
        nc.vector.tensor_scalar(out=escale, in0=nrbw_b, scalar1=-2.0,
                                scalar2=None, op0=Alu.mult)
        ebias = smal.tile([128, 1], f32, tag="ebias")
        nc.vector.scalar_tensor_tensor(out=ebias, in0=A, scalar=16.0,
                                       in1=nrbw_b, op0=Alu.add, op1=Alu.mult)

        # ---- delta broadcast tile ----
        dlt = bigp.tile([128, F], bf16, tag="dlt")
        nc.scalar.dma_start(
            out=dlt, in_=bass.AP(tensor=delta_d, offset=0,
                                 ap=[[F, S], [0, Bc], [1, F]]))

        # ACT observer of ebias/escale (one wait slot per instruction)
        obs2 = smal.tile([128, 1], f32, tag="obs2")
        nc.scalar.copy(out=obs2, in_=ebias)

        # ---- G matmuls + exp + weights scan ----
        weights = bigp.tile([128, F], f32, tag="weights")
        scanW = bigp.tile([128, F], f32, tag="scanW")
        GB = [0, 256, 768, 1280, 1792, 2048]
        NG = len(GB) - 1
        g_ps = [gps.tile([128, GB[j + 1] - GB[j]], f32, name=f"g{j}", tag=f"g{j}")
                for j in range(NG)]
        for j in range(NG):
            for s in range(S):
                hs = H[:, s * F + GB[j]: s * F + GB[j + 1]]
                nc.tensor.matmul(g_ps[j][s * Bc:(s + 1) * Bc, :], Phi, hs,
                                 start=True, stop=False,
                                 tile_position=(0, s * Bc))
                nc.tensor.matmul(g_ps[j][s * Bc:(s + 1) * Bc, :], Plo, hs,
                                 start=False, stop=True,
                                 tile_position=(0, s * Bc))
        for j in range(NG):
            nc.scalar.activation(out=weights[:, GB[j]:GB[j + 1]],
                                 in_=g_ps[j], func=Act.Exp, bias=ebias,
                                 scale=escale)
            nc.vector.tensor_tensor_scan(
                out=scanW[:, GB[j]:GB[j + 1]],
                data0=weights[:, GB[j]:GB[j + 1]],
                data1=weights[:, GB[j]:GB[j + 1]],
                initial=0.0 if j == 0 else scanW[:, GB[j] - 1: GB[j]],
                op0=Alu.add, op1=Alu.bypass)

        # ---- global row sums -> invm ; per-seg offsets of the scan ----
        srep_ps = sps.tile([128, 1], f32, tag="sp")
        nc.tensor.matmul(srep_ps, M2, scanW[:, F - 1:F], start=True, stop=True)
        ssafe = smal.tile([128, 1], f32, tag="ssafe")
        nc.vector.tensor_scalar(out=ssafe, in0=srep_ps, scalar1=float(EPS),
                                scalar2=None, op0=Alu.max)
        sinv = smal.tile([128, 1], f32, tag="sinv")
        nc.vector.reciprocal(out=sinv, in_=ssafe)
        smask = smal.tile([128, 1], f32, tag="smask")
        nc.vector.tensor_scalar(out=smask, in0=srep_ps, scalar1=float(EPS),
                                scalar2=None, op0=Alu.is_ge)
        invm = smal.tile([128, 1], f32, tag="invm")
        nc.vector.tensor_tensor(out=invm, in0=sinv, in1=smask, op=Alu.mult)
        invm_neg = smal.tile([128, 1], f32, tag="invm_neg")
        nc.vector.tensor_scalar(out=invm_neg, in0=invm, scalar1=-1.0,
                                scalar2=None, op0=Alu.mult)

        offsW_ps = sps.tile([128, 1], f32, tag="sp")
        nc.tensor.matmul(offsW_ps, M1, scanW[:, F - 1:F], start=True, stop=True)
        offsW = smal.tile([128, 1], f32, tag="offsWs")
        nc.scalar.copy(out=offsW, in_=offsW_ps)

        # ---- main per-chunk pipeline ----
        wc = bigp.tile([128, F], f32, tag="wc")
        m = bigp.tile([128, F], f32, tag="m")
        shifted = bigp.tile([128, F], f32, tag="shifted")
        shz = bigp.tile([128, F], f32, tag="shz")
        wcz = bigp.tile([128, F], f32, tag="wcz")
        l1 = bigp.tile([128, F], f32, tag="l1")
        l2 = bigp.tile([128, F], f32, tag="l2")
        hz = bigp.tile([128, F], f32, tag="hz")
        surv_raw = bigp.tile([128, F], f32, tag="survraw")
        steps_raw = bigp.tile([128, F], f32, tag="stepsraw")
        surv_out = bigp.tile([128, F], f32, tag="survout")
        steps_out = bigp.tile([128, F], f32, tag="stepsout")

        BNDS = [0, 256, 768, 1280, 1792, 2048]
        def ck_(t, j):
            return t[:, BNDS[j]:BNDS[j + 1]]

        for j in range(len(BNDS) - 1):
            nc.vector.tensor_scalar(out=ck_(wc, j), in0=ck_(scanW, j),
                                    scalar1=offsW, scalar2=invm,
                                    op0=Alu.add, op1=Alu.mult)
            nc.vector.scalar_tensor_tensor(out=ck_(m, j), in0=ck_(wc, j),
                                           scalar=float(C1MTOL), in1=ck_(dlt, j),
                                           op0=Alu.is_lt, op1=Alu.mult)
            nc.vector.scalar_tensor_tensor(out=ck_(shifted, j),
                                           in0=ck_(weights, j),
                                           scalar=invm_neg, in1=ck_(wc, j),
                                           op0=Alu.mult, op1=Alu.add)
            tt_eng = nc.gpsimd if j < 4 else nc.vector
            tt_eng.tensor_tensor(out=ck_(shz, j), in0=ck_(shifted, j),
                                 in1=ck_(m, j), op=Alu.mult)
            tt_eng.tensor_tensor(out=ck_(wcz, j), in0=ck_(wc, j),
                                 in1=ck_(m, j), op=Alu.mult)
            nc.scalar.activation(out=ck_(l1, j), in_=ck_(shz, j), func=Act.Ln,
                                 bias=1.0, scale=-1.0)
            nc.scalar.activation(out=ck_(l2, j), in_=ck_(wcz, j), func=Act.Ln,
                                 bias=1.0, scale=-1.0)
            nc.vector.tensor_tensor_scan(
                out=ck_(hz, j), data0=ck_(l1, j), data1=ck_(l2, j),
                initial=0.0 if j == 0 else hz[:, BNDS[j] - 1: BNDS[j]],
                op0=Alu.add, op1=Alu.subtract)
            nc.scalar.activation(out=ck_(surv_raw, j), in_=ck_(hz, j),
                                 func=Act.Exp, scale=-1.0)
            if j == 0:
                nc.gpsimd.tensor_tensor(out=steps_raw[:, 1:BNDS[1]],
                                        in0=surv_raw[:, 0:BNDS[1] - 1],
                                        in1=surv_raw[:, 1:BNDS[1]], op=Alu.subtract)
            else:
                nc.gpsimd.tensor_tensor(
                    out=ck_(steps_raw, j),
                    in0=surv_raw[:, BNDS[j] - 1:BNDS[j + 1] - 1],
                    in1=ck_(surv_raw, j), op=Alu.subtract)

        # ---- global epilogue scalars ----
        offs2_ps = sps.tile([128, 1], f32, tag="sp")
        nc.tensor.matmul(offs2_ps, M1, hz[:, F - 1:F], start=True, stop=True)
        e2 = smal.tile([128, 1], f32, tag="e2")
        nc.scalar.activation(out=e2, in_=offs2_ps, func=Act.Exp, scale=-1.0)

        lct = smal.tile([128, 1], f32, tag="lct")
        nc.vector.tensor_tensor(out=lct, in0=surv_raw[:, F - 1:F], in1=e2,
                                op=Alu.mult)
        glast_ps = sps.tile([128, 1], f32, tag="sp")
        nc.tensor.matmul(glast_ps, M4, lct, start=True, stop=True)
        s2 = smal.tile([128, 1], f32, tag="s2")
        nc.vector.tensor_scalar(out=s2, in0=glast_ps, scalar1=-1.0, scalar2=1.0,
                                op0=Alu.mult, op1=Alu.add)
        s2safe = smal.tile([128, 1], f32, tag="s2safe")
        nc.vector.tensor_scalar(out=s2safe, in0=s2, scalar1=float(EPS),
                                scalar2=None, op0=Alu.max)
        s2inv = smal.tile([128, 1], f32, tag="s2inv")
        nc.vector.reciprocal(out=s2inv, in_=s2safe)
        s2mask = smal.tile([128, 1], f32, tag="s2mask")
        nc.vector.tensor_scalar(out=s2mask, in0=s2, scalar1=float(EPS),
                                scalar2=None, op0=Alu.is_ge)
        rs2 = smal.tile([128, 1], f32, tag="rs2")
        nc.vector.tensor_tensor(out=rs2, in0=s2inv, in1=s2mask, op=Alu.mult)
        scal = smal.tile([128, 1], f32, tag="scal")
        nc.vector.tensor_tensor(out=scal, in0=e2, in1=rs2, op=Alu.mult)

        pl_ps = sps.tile([128, 1], f32, tag="sp")
        nc.tensor.matmul(pl_ps, M3, lct, start=True, stop=True)
        pl = smal.tile([128, 1], f32, tag="pls")
        nc.vector.tensor_scalar(out=pl, in0=pl_ps, scalar1=e0col, scalar2=None,
                                op0=Alu.add)

        sv0 = smal.tile([128, 1], f32, tag="sv0")
        nc.vector.tensor_tensor(out=sv0, in0=surv_raw[:, 0:1], in1=e2,
                                op=Alu.mult)
        st0 = smal.tile([128, 1], f32, tag="st0")
        nc.vector.tensor_tensor(out=st0, in0=pl, in1=sv0, op=Alu.subtract)
        nc.vector.tensor_scalar(out=steps_out[:, 0:1], in0=st0, scalar1=rs2,
                                scalar2=None, op0=Alu.mult)

        # ---- final scaling + DMA out ----
        for j in range(len(BNDS) - 1):
            w_ = BNDS[j + 1] - BNDS[j]
            nc.vector.tensor_scalar(out=ck_(surv_out, j), in0=ck_(surv_raw, j),
                                    scalar1=e2, scalar2=None, op0=Alu.mult)
            if j == 0:
                nc.scalar.mul(out=steps_out[:, 1:BNDS[1]],
                              in_=steps_raw[:, 1:BNDS[1]], mul=scal)
            else:
                nc.scalar.mul(out=ck_(steps_out, j),
                              in_=ck_(steps_raw, j), mul=scal)
            sv_dst = bass.AP(tensor=surv_d, offset=BNDS[j],
                             ap=[[F, S], [N, Bc], [1, w_]])
            st_dst = bass.AP(tensor=steps_d, offset=BNDS[j],
                             ap=[[F, S], [N, Bc], [1, w_]])
            dma(out=sv_dst, in_=surv_out[:, BNDS[j]:BNDS[j + 1]])
            nc.scalar.dma_start(out=st_dst, in_=steps_out[:, BNDS[j]:BNDS[j + 1]])

    # Prefer the activation table containing BOTH Exp and Ln so the whole
    # kernel needs a single table load.
    import concourse.bacc as _bacc_mod
    import concourse.hw_specs as _hw
    _orig_get = _hw.get_activation_tables

    def _filtered(arch):
        t = dict(_orig_get(arch))
        pref = [k for k in t if "natural_log_exp" in k]
        if not pref:
            return t
        exp_ln = {f for f in t[pref[0]]
                  if getattr(f, "name", str(f)) in ("Exp", "Ln")}
        out = {}
        for k, fns in t.items():
            out[k] = set(fns) if k in pref else set(fns) - exp_ln
        return out

    _bacc_mod.get_activation_tables = _filtered
    try:
        nc.compile()
    finally:
        _bacc_mod.get_activation_tables = _orig_get
    return nc


def make_in_maps(c_p, c_in, delta_in, bandwidth):
    c_p = np.asarray(c_p, np.float32)
    c_in = np.asarray(c_in)
    delta_in = np.asarray(delta_in, np.float32)
    bandwidth = np.asarray(bandwidth, np.float32)

    cinT = np.ascontiguousarray(c_in.T).astype(ml_dtypes.bfloat16)   # [C, N]
    delta_bf = delta_in.astype(ml_dtypes.bfloat16)                   # [N]
    band = bandwidth.reshape(1, 1)

    in_maps = []
    for core in range(NCORES):
        b0 = core * Bc
        cp_local = c_p[:, b0:b0 + Bc, :]                             # [C, Bc, K]
        cpT = np.ascontiguousarray(
            cp_local.transpose(2, 0, 1).reshape(CK, Bc))             # [k*16+c, b]
        cpB = np.ascontiguousarray(
            cp_local.transpose(1, 0, 2).reshape(Bc, CK))             # [b, c*8+k]
        in_maps.append({
            "cpT": cpT, "cpB": cpB, "cinT": cinT,
            "delta": delta_bf, "band": band,
        })
    return in_maps


_CACHED_NC = None
_CACHED_RUN = None


def _get_nc():
    global _CACHED_NC
    if _CACHED_NC is None:
        _CACHED_NC = build_nc()
    return _CACHED_NC


def _get_runner():
    """Build (once) a cached sharded jit callable over the 8 cores."""
    global _CACHED_RUN
    if _CACHED_RUN is not None:
        return _CACHED_RUN
    import jax
    from jax.sharding import Mesh, PartitionSpec
    from jax.experimental.shard_map import shard_map
    import concourse.mybir as mb
    from concourse import bass2jax
    from concourse.bass2jax import (_bass_exec_p, install_neuronx_cc_hook,
                                    partition_id_tensor)

    nc = _get_nc()
    install_neuronx_cc_hook()

    pid_name = nc.partition_id_tensor.name if nc.partition_id_tensor else None
    in_names, out_names, out_avals, zero_shapes = [], [], [], []
    for alloc in nc.m.functions[0].allocations:
        if not isinstance(alloc, mb.MemoryLocationSet):
            continue
        if not alloc.memorylocations:
            continue
        name = alloc.memorylocations[0].name
        if alloc.kind == "ExternalInput":
            if name == pid_name:
                continue
            in_names.append(name)
        elif alloc.kind == "ExternalOutput":
            out_names.append(name)
            shape = tuple(alloc.tensor_shape)
            dtype = mb.dt.np(alloc.dtype)
            out_avals.append(jax.core.ShapedArray(shape, dtype))
            zero_shapes.append((shape, dtype))
    n_params = len(in_names)
    all_names = in_names + out_names
    if pid_name is not None:
        all_names = all_names + [pid_name]
    donate = tuple(range(n_params, n_params + len(out_names)))

    def _body(*args):
        operands = list(args)
        if pid_name is not None:
            operands.append(partition_id_tensor())
        outs = _bass_exec_p.bind(
            *operands, out_avals=tuple(out_avals), in_names=tuple(all_names),
            out_names=tuple(out_names), lowering_input_output_aliases=(),
            sim_require_finite=True, sim_require_nnan=True, nc=nc)
        return tuple(outs)

    devices = jax.devices()[:NCORES]
    mesh = Mesh(np.asarray(devices), ("core",))
    specs = (PartitionSpec("core"),) * (n_params + len(out_names))
    out_specs = (PartitionSpec("core"),) * len(out_names)
    sharded = jax.jit(
        shard_map(_body, mesh=mesh, in_specs=specs, out_specs=out_specs,
                  check_rep=False),
        donate_argnums=donate, keep_unused=True)

    def run(in_maps):
        concat_in = [
            np.concatenate([np.asarray(im[name]) for im in in_maps], axis=0)
            for name in in_names]
        concat_zeros = [
            np.zeros((NCORES * sh[0], *sh[1:]), dt) for sh, dt in zero_shapes]
        out = sharded(*concat_in, *concat_zeros)
        res = {}
        for i, name in enumerate(out_names):
            res[name] = np.asarray(out[i])  # [NCORES*Bc, N]
        return res

    _CACHED_RUN = run
    return run


def kernel(c_p, c_in, delta_in, bandwidth):
    in_maps = make_in_maps(c_p, c_in, delta_in, bandwidth)
    run = _get_runner()
    res = run(in_maps)
    return res["surv"], res["steps"]


if __name__ == "__main__":
    rng = np.random.default_rng(0)
    c_p = rng.standard_normal((C, B, K), dtype=np.float32)
    c_in = rng.integers(0, K, size=(N, C)).astype(np.int32)
    delta = (rng.random(N) > 0.3).astype(np.float32)
    band = np.ones((1,), np.float32)
    import time
    t0 = time.time()
    sf, ss = kernel(c_p=c_p, c_in=c_in, delta_in=delta, bandwidth=band)
    print("first call", time.time() - t0, "s", sf.shape, ss.shape,
          float(sf.sum()), float(ss.sum()))
    t0 = time.time()
    sf, ss = kernel(c_p=c_p, c_in=c_in, delta_in=delta, bandwidth=band)
    print("second call", time.time() - t0, "s")

